# revision 12
# baseline (speedup 1.0000x reference)
"""Trainium2 Bass kernel for nn_Attention_32049045963483 (sparse_attention).

Math collapse (verified vs reference: ~3e-6 rel err fp32, ~5e-3 with the
fp16 + sigmoid-form-gelu device pipeline; gate is 2e-2):
  - qkv 1x1 conv folds into the 11x11/stride-8 down-convs (host-side fold):
      w_eff[d, ky, kx, oc] = sum_ic w[oc,ic,ky,kx] W1[ic,d]
  - nearest upsample-by-64 + softmax == softmax of the low-res [64,64] map;
    output row X depends only on low-res index x = X.
  - v enters only through 64-wide block sums: vbar = Wv @ fbar,
      fbar[d,J] = sum_y f[d,J,y]
  - out[c,X,Y] = (sum_J e[J,X] vbar[c,J]) / (64 * sum_J e[J,X]),
      e[J,I] = exp(scale * q_I . k_J), broadcast along Y.

Device kernel (per core = one head), all fp16 on the PE:
  - one dma_start per HWDGE ring (extra dma_starts stall the engine ring
    ~1.2us on the trailing sem-write descriptor); 4KB descriptors only
    (6KB/8KB descriptors measured ~2x slower per SDMA engine) -- the s2
    stream lands in two chunks split by a 128-col SBUF gap so the lowering
    cannot coalesce them.  SDMA engines round-robin rings at packet
    granularity, so per-engine bytes are the bandwidth cap.
  - conv as 121 per-tap matmuls into ONE [16,64] psum region (q ch 0-7,
    k ch 8-15; stationary w_eff[:, tap, 16] fp16, moving = strided f
    slice; no padding -- border taps use restricted oy/ox ranges), plus a
    final ones-row matmul in the same accumulation group that adds the
    biases.
  - PE warm-up: dummy bf16 matmuls during the DMA wait flip the HAM clock
    gate to 2.4 GHz before the real work.
  - fbar reduced on DVE in fp16 while PE does the conv (hidden).
  - 2*gelu(x) ~= x*(1+tanh(0.851x)) straight off the conv psum: one ACT
    Tanh + one DVE scalar_tensor_tensor (the 0.5 of both gelus folds into
    the exp scale).  ACT only ever needs the exp_and_others table: no
    mid-kernel ACT table reload.  k is then moved to base-partition 0 via
    a [16,8] selector matmul (PE partition slices must be 32-aligned).
  - contiguous [64,512] store split across both rings; host reorders
    [x, c, y] -> [c, x, y].
"""

import numpy as np

N_CORES = 8
SCALE = 8.0 ** -0.5  # dim_head ** -0.5

_CACHE = {}
LAST_RESULTS = None  # BassKernelResults of the most recent run (for test harness)

# tap order: a full-rectangle interior tap first (the start=True matmul
# covers the full [16,64] region)
_HEAD_TAPS = [(5, 3)]
TAPS = _HEAD_TAPS + [
    (ky, kx)
    for ky in range(11)
    for kx in range(11)
    if (ky, kx) not in _HEAD_TAPS
]

N_DUMMY = 12  # HAM warm-up matmuls
CW = 4224  # big_t column offset of the w_eff pack (128-col gap after f)


def _rng(kidx):
    """Valid output range [o0, o1) and first input row for kernel offset."""
    o0 = 1 if kidx < 2 else 0
    o1 = 7 if kidx == 10 else 8
    r0 = 8 * o0 + kidx - 2
    return o0, o1, r0


def _dep(after, before, sync=False):
    from concourse.tile import add_dep_helper

    a = getattr(after, "ins", after)
    b = getattr(before, "ins", before)
    add_dep_helper(a, b, sync=sync, reason="pin order")


def _build_nc():
    from contextlib import ExitStack

    import concourse.bacc as bacc
    import concourse.bass as bass
    import concourse.mybir as mybir
    import concourse.tile as tile

    f32 = mybir.dt.float32
    f16 = mybir.dt.float16
    bf16 = mybir.dt.bfloat16
    X = mybir.AxisListType.X
    AF = mybir.ActivationFunctionType
    ALU = mybir.AluOpType

    nc = bacc.Bacc("TRN2", target_bir_lowering=False)

    s1_d = nc.dram_tensor("s1", [64, 2048], f16, kind="ExternalInput")
    s2_d = nc.dram_tensor("s2", [64, 4096], f16, kind="ExternalInput")
    wsm_d = nc.dram_tensor("wsm", [16, 88], f16, kind="ExternalInput")
    out_d = nc.dram_tensor("out", [64, 512], f32, kind="ExternalOutput")

    with tile.TileContext(nc) as tc:
        with ExitStack() as ctx:
            sb = ctx.enter_context(tc.tile_pool(name="sb", bufs=1))
            ps = ctx.enter_context(tc.tile_pool(name="ps", bufs=1, space="PSUM"))

            big_t = sb.tile([64, 6272], f16)  # f | gap | w_eff pack | pad
            wsm_t = sb.tile([16, 88], f16)  # sel | biases | ones
            fbar_t = sb.tile([64, 64], f16)
            h2_t = sb.tile([16, 64], f16)
            qk_t = sb.tile([16, 64], f16)
            k_t = sb.tile([8, 64], f16)
            e_t = sb.tile([64, 64], f16)
            vaug_t = sb.tile([64, 9], f16)
            rs_t = sb.tile([64, 1], f32)
            olT_t = sb.tile([64, 8], f32)
            T_t = sb.tile([64, 8 * 64], f32)
            scr_t = sb.tile([1, 1], f32)
            scr2_t = sb.tile([1, 1], f32)
            dmw_t = sb.tile([64, 128], bf16)
            dmx_t = sb.tile([64, 256], bf16)

            # --- one DMA per ring; s2 = [f-half | w_eff] in two 4KB chunks
            nc.sync.dma_start(out=big_t[:, 0:2048], in_=s1_d[:])
            dst2 = bass.AP(
                tensor=big_t.tensor, offset=big_t[:, 2048:2049].offset,
                ap=[list(big_t[:].ap[0]), [2176, 2], [1, 2048]],
            )
            d_s2 = nc.scalar.dma_start(
                out=dst2,
                in_=s2_d[:].rearrange("p (c k) -> p c k", k=2048),
            )
            nc.sync.dma_start(out=wsm_t, in_=wsm_d[:])

            # --- DVE constants + dummy sources (run during DMA wait)
            m_dw = nc.vector.memset(dmw_t, 0.0)
            m_dx = nc.vector.memset(dmx_t, 0.0)
            nc.vector.memset(vaug_t[:, 8:9], 64.0)
            nc.vector.memset(scr_t, 0.0)

            # --- ACT exp-table preload AFTER the scalar-ring DMA trigger
            de = nc.scalar.activation(out=scr2_t, in_=scr_t, func=AF.Exp)
            _dep(de, d_s2)

            # --- HAM warm-up: dummy bf16 matmuls keep PE busy ~3.4us so the
            # clock gate opens to 2.4 GHz before the conv starts
            pd_t = ps.tile([128, 256], f32, tag="E")
            dmy = None
            for i in range(N_DUMMY):
                dmy = nc.tensor.matmul(
                    pd_t, dmw_t, dmx_t, start=True, stop=True,
                    skip_group_check=True,
                )
                if i == 0:
                    _dep(dmy, m_dw)
                    _dep(dmy, m_dx)

            # --- fbar[d, x] = sum_y f[d, x, y] on DVE (fp16, overlaps conv)
            f3 = big_t[:, 0:4096].rearrange("p (x y) -> p x y", y=64)
            with nc.allow_low_precision("fp16 block-sum; 2e-2 rel-err budget"):
                for j in range(4):
                    rd = nc.vector.reduce_sum(
                        out=fbar_t[:, 16 * j : 16 * (j + 1)],
                        in_=f3[:, 16 * j : 16 * j + 16, :],
                        axis=X,
                    )
                    if j >= 2:
                        # cols 2048+ land via the hand-built dst2 AP whose
                        # region the dep tracker may under-approximate
                        _dep(rd, d_s2, sync=True)

            # --- conv: 121 taps, one [16,64] accumulation region, then the
            # bias via a ones-row matmul in the same group
            pc = ps.tile([16, 64], f32, tag="A")
            pc4 = pc.rearrange("p (x y) -> p x y", y=8)
            for t_i, (ky, kx) in enumerate(TAPS):
                oy0, oy1, ry0 = _rng(ky)
                ox0, ox1, cx0 = _rng(kx)
                n_oy, n_ox = oy1 - oy0, ox1 - ox0
                rhs = f3[
                    :,
                    ry0 : ry0 + 8 * (n_oy - 1) + 1 : 8,
                    cx0 : cx0 + 8 * (n_ox - 1) + 1 : 8,
                ]
                mm = nc.tensor.matmul(
                    pc4[:, oy0:oy1, ox0:ox1],
                    big_t[:, CW + 16 * t_i : CW + 16 * t_i + 16],
                    rhs,
                    start=(t_i == 0),
                    stop=False,
                )
                if t_i == 0:
                    _dep(mm, dmy)
                    _dep(mm, d_s2, sync=True)
            nc.tensor.matmul(
                pc, wsm_t[0:1, 8:24], wsm_t[0:1, 24:88],
                start=False, stop=True,
            )

            # --- vbar: vaug[J, c] = sum_d fbar[d,J] wvt[d,c]
            psv = ps.tile([64, 8], f32, tag="B")
            nc.tensor.matmul(
                psv, fbar_t, big_t[:, CW + 1936 : CW + 1944],
                start=True, stop=True,
            )
            nc.scalar.copy(out=vaug_t[:, 0:8], in_=psv)

            # --- 2*gelu(x) ~= x*(1+tanh(0.851x)) straight off the conv psum
            nc.scalar.activation(out=h2_t, in_=pc, func=AF.Tanh, scale=0.851)
            nc.vector.scalar_tensor_tensor(
                out=qk_t, in0=h2_t, scalar=1.0, in1=pc,
                op0=ALU.add, op1=ALU.mult,
            )

            # --- k to base partition 0 via selector matmul + ACT copy
            pss = ps.tile([8, 64], f32, tag="F")
            nc.tensor.matmul(pss, wsm_t[:, 0:8], qk_t, start=True, stop=True)
            nc.scalar.copy(out=k_t, in_=pss)

            # --- dots_T[J, I] = sum_c k[c,J] q[c,I]; e = exp(S/4 * dots_T)
            psd = ps.tile([64, 64], f32, tag="H")
            nc.tensor.matmul(psd, k_t, qk_t[0:8, :], start=True, stop=True)
            nc.scalar.activation(out=e_t, in_=psd, func=AF.Exp, scale=SCALE / 4)

            # --- out_u[I, 0:8] = sum_J e[J,I] vaug[J,c]; col 8 = 64*sum_J e
            pso = ps.tile([64, 9], f32, tag="C")
            nc.tensor.matmul(pso, e_t, vaug_t, start=True, stop=True)
            nc.vector.reciprocal(out=rs_t, in_=pso[:, 8:9])
            nc.vector.tensor_scalar_mul(olT_t, pso[:, 0:8], rs_t)

            # --- broadcast along y (stride-0 read), split in halves so each
            # ring stores its half as soon as it is ready
            T3 = T_t.rearrange("p (c y) -> p c y", y=64)
            for h, eng in ((0, nc.sync), (1, nc.scalar)):
                ola = olT_t[:, 4 * h : 4 * h + 4]
                ol_b = bass.AP(
                    tensor=ola.tensor, offset=ola.offset,
                    ap=[list(ola.ap[0]), list(ola.ap[1]), [0, 64]],
                )
                nc.vector.tensor_copy(out=T3[:, 4 * h : 4 * h + 4, :], in_=ol_b)
                eng.dma_start(
                    out=out_d[:, 256 * h : 256 * h + 256],
                    in_=T_t[:, 256 * h : 256 * h + 256],
                )

    nc.finalize()
    return nc


def _get_nc():
    if "nc" not in _CACHE:
        _CACHE["nc"] = _build_nc()
    return _CACHE["nc"]


def kernel(**inputs):
    global LAST_RESULTS
    from concourse.bass_utils import run_bass_kernel_spmd

    f = np.ascontiguousarray(inputs["f"], np.float32)
    w_qkv = np.ascontiguousarray(inputs["w_qkv"], np.float32)[:, :, 0, 0]
    wq = np.ascontiguousarray(inputs["wq"], np.float32)
    wk = np.ascontiguousarray(inputs["wk"], np.float32)
    bq = np.ascontiguousarray(inputs["bq"], np.float32)
    bk = np.ascontiguousarray(inputs["bk"], np.float32)

    W1q, W1k, Wv = w_qkv[0:64], w_qkv[64:128], w_qkv[128:192]
    # w_eff[ky, kx, oc, d] = sum_ic w[oc, ic, ky, kx] * W1[ic, d]
    weq = np.einsum("oikl,id->klod", wq, W1q).astype(np.float16)
    wek = np.einsum("oikl,id->klod", wk, W1k).astype(np.float16)

    f16 = f[0].reshape(64, 4096).astype(np.float16)
    s1 = np.ascontiguousarray(f16[:, 0:2048])

    in_maps = []
    for i in range(N_CORES):
        sl = slice(8 * i, 8 * i + 8)
        s2 = np.zeros((64, 4096), np.float16)
        s2[:, 0:2048] = f16[:, 2048:4096]
        for t_i, (ky, kx) in enumerate(TAPS):
            s2[:, 2048 + 16 * t_i : 2048 + 16 * t_i + 8] = weq[ky, kx, sl].T
            s2[:, 2048 + 16 * t_i + 8 : 2048 + 16 * t_i + 16] = wek[
                ky, kx, sl
            ].T
        s2[:, 2048 + 1936 : 2048 + 1944] = Wv[sl].T.astype(np.float16)
        wsm = np.zeros((16, 88), np.float16)
        for c in range(8):
            wsm[8 + c, c] = 1.0  # selector: k = qk[8:16]
        wsm[0, 8:16] = bq[sl]
        wsm[0, 16:24] = bk[sl]
        wsm[0, 24:88] = 1.0
        in_maps.append({"s1": s1, "s2": s2, "wsm": wsm})

    nc = _get_nc()
    res = run_bass_kernel_spmd(nc, in_maps, core_ids=list(range(N_CORES)))
    LAST_RESULTS = res
    outs = []
    for r in res.results:
        t = r["out"].reshape(64, 8, 64).transpose(1, 0, 2)  # [c, x, y]
        outs.append(t.reshape(8, 4096))
    out = np.concatenate(outs, axis=0)  # [64, 4096]
    return out.reshape(1, 64, 64, 64)


# revision 13
# speedup vs baseline: 1.3445x; 1.3445x over previous
"""Trainium2 Bass kernel for nn_Attention_32049045963483 (sparse_attention).

Math collapse (verified vs reference: ~3e-6 rel err fp32, ~5e-3 with the
fp16 + sigmoid-form-gelu device pipeline; gate is 2e-2):
  - qkv 1x1 conv folds into the 11x11/stride-8 down-convs (host-side fold):
      w_eff[d, ky, kx, oc] = sum_ic w[oc,ic,ky,kx] W1[ic,d]
  - nearest upsample-by-64 + softmax == softmax of the low-res [64,64] map;
    output row X depends only on low-res index x = X.
  - v enters only through 64-wide block sums: vbar = Wv @ fbar,
      fbar[d,J] = sum_y f[d,J,y]
  - out[c,X,Y] = (sum_J e[J,X] vbar[c,J]) / (64 * sum_J e[J,X]),
      e[J,I] = exp(scale * q_I . k_J), broadcast along Y.

Device kernel (per core = one head), all fp16 on the PE:
  - one dma_start per HWDGE ring (extra dma_starts stall the engine ring
    ~1.2us on the trailing sem-write descriptor); 4KB descriptors only
    (6KB/8KB descriptors measured ~2x slower per SDMA engine) -- the s2
    stream lands in two chunks split by a 128-col SBUF gap so the lowering
    cannot coalesce them.  SDMA engines round-robin rings at packet
    granularity, so per-engine bytes are the bandwidth cap.
  - conv as 121 per-tap matmuls: stationary w_eff[:, tap, 16(q8|k8)] fp16,
    moving = strided f slice; 4-way tile_position column packing with the
    four groups in ONE psum bank at partitions 32g..32g+15 (per-partition
    zero regions).  Packing matters: consecutive matmuls accumulating into
    the SAME region serialize on the psum read-modify-write (~113ns/MM vs
    ~36ns with 4 independent regions).  No input padding: border taps use
    restricted oy/ox ranges.
  - PE warm-up: dummy bf16 matmuls during the DMA wait flip the HAM clock
    gate to 2.4 GHz before the real work.
  - fbar reduced on DVE in fp16 while PE does the conv (hidden).
  - q|k fused in one [8,128] psum tile (one accumulation group, 3 matmuls:
    Eq-combine, Ek-combine, bias via a 2-row indicator matmul); then
    2*gelu(x) ~= x*(1+tanh(0.851x)): one ACT Tanh + one DVE
    scalar_tensor_tensor (the 0.5 of both gelus folds into the exp
    scale).  ACT only ever needs the exp_and_others table (tanh+exp): no
    mid-kernel ACT table reload.
  - contiguous [64,512] store split across both rings; host reorders
    [x, c, y] -> [c, x, y].
"""

import numpy as np

N_CORES = 8
SCALE = 8.0 ** -0.5  # dim_head ** -0.5

_CACHE = {}
LAST_RESULTS = None  # BassKernelResults of the most recent run (for test harness)

# tap order: 4 full-rectangle interior taps first (one per column group, so
# each group's start=True matmul covers its full [16,64] region)
_HEAD_TAPS = [(5, 3), (5, 4), (5, 5), (5, 6)]
TAPS = _HEAD_TAPS + [
    (ky, kx)
    for ky in range(11)
    for kx in range(11)
    if (ky, kx) not in _HEAD_TAPS
]

N_DUMMY = 12  # HAM warm-up matmuls
CW = 4224  # big_t column offset of the w_eff pack (128-col gap after f)


def _rng(kidx):
    """Valid output range [o0, o1) and first input row for kernel offset."""
    o0 = 1 if kidx < 2 else 0
    o1 = 7 if kidx == 10 else 8
    r0 = 8 * o0 + kidx - 2
    return o0, o1, r0


def _dep(after, before, sync=False):
    from concourse.tile import add_dep_helper

    a = getattr(after, "ins", after)
    b = getattr(before, "ins", before)
    add_dep_helper(a, b, sync=sync, reason="pin order")


def _build_nc():
    from contextlib import ExitStack

    import concourse.bacc as bacc
    import concourse.bass as bass
    import concourse.mybir as mybir
    import concourse.tile as tile

    f32 = mybir.dt.float32
    f16 = mybir.dt.float16
    bf16 = mybir.dt.bfloat16
    X = mybir.AxisListType.X
    AF = mybir.ActivationFunctionType
    ALU = mybir.AluOpType

    nc = bacc.Bacc("TRN2", target_bir_lowering=False)

    s1_d = nc.dram_tensor("s1", [64, 2048], f16, kind="ExternalInput")
    s2_d = nc.dram_tensor("s2", [64, 4096], f16, kind="ExternalInput")
    wE_d = nc.dram_tensor("wE", [128, 16], f16, kind="ExternalInput")
    wc_d = nc.dram_tensor("wc", [2, 136], f16, kind="ExternalInput")
    out_d = nc.dram_tensor("out", [64, 512], f32, kind="ExternalOutput")

    with tile.TileContext(nc) as tc:
        with ExitStack() as ctx:
            sb = ctx.enter_context(tc.tile_pool(name="sb", bufs=1))
            ps = ctx.enter_context(tc.tile_pool(name="ps", bufs=1, space="PSUM"))

            big_t = sb.tile([64, 6272], f16)  # f | gap | w_eff pack | pad
            wE_t = sb.tile([128, 16], f16)
            wc_t = sb.tile([2, 136], f16)
            fbar_t = sb.tile([64, 64], f16)
            S_t = sb.tile([128, 64], f16)
            h2_t = sb.tile([8, 128], f16)
            qk_t = sb.tile([8, 128], f16)
            e_t = sb.tile([64, 64], f16)
            vaug_t = sb.tile([64, 9], f16)
            rs_t = sb.tile([64, 1], f32)
            olT_t = sb.tile([64, 8], f32)
            T_t = sb.tile([64, 8 * 64], f32)
            scr_t = sb.tile([1, 1], f32)
            scr2_t = sb.tile([1, 1], f32)
            dmw_t = sb.tile([64, 128], bf16)
            dmx_t = sb.tile([64, 256], bf16)

            # --- one DMA per ring; s2 = [f-half | w_eff] in two 4KB chunks
            nc.sync.dma_start(out=big_t[:, 0:2048], in_=s1_d[:])
            dst2 = bass.AP(
                tensor=big_t.tensor, offset=big_t[:, 2048:2049].offset,
                ap=[list(big_t[:].ap[0]), [2176, 2], [1, 2048]],
            )
            d_s2 = nc.scalar.dma_start(
                out=dst2,
                in_=s2_d[:].rearrange("p (c k) -> p c k", k=2048),
            )
            nc.sync.dma_start(out=wE_t, in_=wE_d[:])
            nc.sync.dma_start(out=wc_t, in_=wc_d[:])

            # --- DVE constants + dummy sources (run during DMA wait)
            m_dw = nc.vector.memset(dmw_t, 0.0)
            m_dx = nc.vector.memset(dmx_t, 0.0)
            nc.vector.memset(vaug_t[:, 8:9], 64.0)
            nc.vector.memset(scr_t, 0.0)

            # --- conv accumulator: ONE psum bank; zero it so the 16-row gaps
            # the column groups leave read back clean in the single S-copy
            pc = ps.tile([128, 64], f32, tag="A")
            pc4 = pc.rearrange("p (x y) -> p x y", y=8)
            gap_ms = [nc.vector.memset(pc, 0.0)]

            # --- ACT exp-table preload AFTER the scalar-ring DMA trigger
            de = nc.scalar.activation(out=scr2_t, in_=scr_t, func=AF.Exp)
            _dep(de, d_s2)

            # --- HAM warm-up: dummy bf16 matmuls keep PE busy ~3.4us so the
            # clock gate opens to 2.4 GHz before the conv starts
            pd_t = ps.tile([128, 256], f32, tag="E")
            dmy = None
            for i in range(N_DUMMY):
                dmy = nc.tensor.matmul(
                    pd_t, dmw_t, dmx_t, start=True, stop=True,
                    skip_group_check=True,
                )
                if i == 0:
                    _dep(dmy, m_dw)
                    _dep(dmy, m_dx)

            # --- fbar[d, x] = sum_y f[d, x, y] on DVE (fp16, overlaps conv)
            f3 = big_t[:, 0:4096].rearrange("p (x y) -> p x y", y=64)
            with nc.allow_low_precision("fp16 block-sum; 2e-2 rel-err budget"):
                for j in range(4):
                    rd = nc.vector.reduce_sum(
                        out=fbar_t[:, 16 * j : 16 * (j + 1)],
                        in_=f3[:, 16 * j : 16 * j + 16, :],
                        axis=X,
                    )
                    if j >= 2:
                        # cols 2048+ land via the hand-built dst2 AP whose
                        # region the dep tracker may under-approximate
                        _dep(rd, d_s2, sync=True)

            # --- conv: 121 taps, 4-way column packing, PSUM accumulate
            totals = [len(range(g, 121, 4)) for g in range(4)]
            seen = [0, 0, 0, 0]
            for t_i, (ky, kx) in enumerate(TAPS):
                g = t_i % 4
                oy0, oy1, ry0 = _rng(ky)
                ox0, ox1, cx0 = _rng(kx)
                n_oy, n_ox = oy1 - oy0, ox1 - ox0
                rhs = f3[
                    :,
                    ry0 : ry0 + 8 * (n_oy - 1) + 1 : 8,
                    cx0 : cx0 + 8 * (n_ox - 1) + 1 : 8,
                ]
                outap = pc4[32 * g : 32 * g + 16, oy0:oy1, ox0:ox1]
                seen[g] += 1
                mm = nc.tensor.matmul(
                    outap,
                    big_t[:, CW + 16 * t_i : CW + 16 * t_i + 16],
                    rhs,
                    start=(seen[g] == 1),
                    stop=(seen[g] == totals[g]),
                    tile_position=(0, 32 * g),
                )
                if t_i == 0:
                    _dep(mm, dmy)
                    _dep(mm, d_s2, sync=True)
                    for gm in gap_ms:
                        _dep(mm, gm)

            # --- single PSUM -> SBUF copy of all conv partials
            nc.vector.tensor_copy(out=S_t, in_=pc)

            # --- vbar while DVE copies S: vaug[J, c] = sum_d fbar[d,J] wvt[d,c]
            psv = ps.tile([64, 8], f32, tag="B")
            nc.tensor.matmul(
                psv, fbar_t, big_t[:, CW + 1936 : CW + 1944],
                start=True, stop=True,
            )
            nc.scalar.copy(out=vaug_t[:, 0:8], in_=psv)

            # --- combine column groups + bias, q|k fused in one [8,128] bank:
            # cols 0:64 = q + bq, cols 64:128 = k + bk (bias via 2-row matmul)
            psqk = ps.tile([8, 128], f32, tag="F")
            nc.tensor.matmul(
                psqk[:, 0:64], wE_t[:, 0:8], S_t, start=True, stop=False
            )
            nc.tensor.matmul(
                psqk[:, 64:128], wE_t[:, 8:16], S_t, start=False, stop=False
            )
            nc.tensor.matmul(
                psqk, wc_t[:, 0:8], wc_t[:, 8:136], start=False, stop=True
            )

            # --- 2*gelu(x) ~= x*(1+tanh(0.851x)) on [8,128] (sigmoid form,
            # l2 5e-3 vs 2e-2 budget); the 0.5 folds into the exp scale
            nc.scalar.activation(out=h2_t, in_=psqk, func=AF.Tanh, scale=0.851)
            nc.vector.scalar_tensor_tensor(
                out=qk_t, in0=h2_t, scalar=1.0, in1=psqk,
                op0=ALU.add, op1=ALU.mult,
            )

            # --- dots_T[J, I] = sum_c k[c,J] q[c,I]; e = exp(S/4 * dots_T)
            psd = ps.tile([64, 64], f32, tag="H")
            nc.tensor.matmul(
                psd, qk_t[:, 64:128], qk_t[:, 0:64], start=True, stop=True
            )
            nc.scalar.activation(out=e_t, in_=psd, func=AF.Exp, scale=SCALE / 4)

            # --- out_u[I, 0:8] = sum_J e[J,I] vaug[J,c]; col 8 = 64*sum_J e
            pso = ps.tile([64, 9], f32, tag="C")
            nc.tensor.matmul(pso, e_t, vaug_t, start=True, stop=True)
            nc.vector.reciprocal(out=rs_t, in_=pso[:, 8:9])
            nc.vector.tensor_scalar_mul(olT_t, pso[:, 0:8], rs_t)

            # --- broadcast along y (stride-0 read), split in halves so each
            # ring stores its half as soon as it is ready
            T3 = T_t.rearrange("p (c y) -> p c y", y=64)
            for h, eng in ((0, nc.sync), (1, nc.scalar)):
                ola = olT_t[:, 4 * h : 4 * h + 4]
                ol_b = bass.AP(
                    tensor=ola.tensor, offset=ola.offset,
                    ap=[list(ola.ap[0]), list(ola.ap[1]), [0, 64]],
                )
                nc.vector.tensor_copy(out=T3[:, 4 * h : 4 * h + 4, :], in_=ol_b)
                eng.dma_start(
                    out=out_d[:, 256 * h : 256 * h + 256],
                    in_=T_t[:, 256 * h : 256 * h + 256],
                )

    nc.finalize()
    return nc


def _get_nc():
    if "nc" not in _CACHE:
        _CACHE["nc"] = _build_nc()
    return _CACHE["nc"]


def kernel(**inputs):
    global LAST_RESULTS
    from concourse.bass_utils import run_bass_kernel_spmd

    f = np.ascontiguousarray(inputs["f"], np.float32)
    w_qkv = np.ascontiguousarray(inputs["w_qkv"], np.float32)[:, :, 0, 0]
    wq = np.ascontiguousarray(inputs["wq"], np.float32)
    wk = np.ascontiguousarray(inputs["wk"], np.float32)
    bq = np.ascontiguousarray(inputs["bq"], np.float32)
    bk = np.ascontiguousarray(inputs["bk"], np.float32)

    W1q, W1k, Wv = w_qkv[0:64], w_qkv[64:128], w_qkv[128:192]
    # w_eff[ky, kx, oc, d] = sum_ic w[oc, ic, ky, kx] * W1[ic, d]
    weq = np.einsum("oikl,id->klod", wq, W1q).astype(np.float16)
    wek = np.einsum("oikl,id->klod", wk, W1k).astype(np.float16)

    f16 = f[0].reshape(64, 4096).astype(np.float16)
    s1 = np.ascontiguousarray(f16[:, 0:2048])

    in_maps = []
    for i in range(N_CORES):
        sl = slice(8 * i, 8 * i + 8)
        s2 = np.zeros((64, 4096), np.float16)
        s2[:, 0:2048] = f16[:, 2048:4096]
        for t_i, (ky, kx) in enumerate(TAPS):
            s2[:, 2048 + 16 * t_i : 2048 + 16 * t_i + 8] = weq[ky, kx, sl].T
            s2[:, 2048 + 16 * t_i + 8 : 2048 + 16 * t_i + 16] = wek[
                ky, kx, sl
            ].T
        s2[:, 2048 + 1936 : 2048 + 1944] = Wv[sl].T.astype(np.float16)
        wE = np.zeros((128, 16), np.float16)
        for g in range(4):
            for c in range(8):
                wE[32 * g + c, c] = 1.0
                wE[32 * g + 8 + c, 8 + c] = 1.0
        wc = np.zeros((2, 136), np.float16)
        wc[0, 0:8] = bq[sl]
        wc[1, 0:8] = bk[sl]
        wc[0, 8 : 8 + 64] = 1.0
        wc[1, 8 + 64 : 8 + 128] = 1.0
        in_maps.append({"s1": s1, "s2": s2, "wE": wE, "wc": wc})

    nc = _get_nc()
    res = run_bass_kernel_spmd(nc, in_maps, core_ids=list(range(N_CORES)))
    LAST_RESULTS = res
    outs = []
    for r in res.results:
        t = r["out"].reshape(64, 8, 64).transpose(1, 0, 2)  # [c, x, y]
        outs.append(t.reshape(8, 4096))
    out = np.concatenate(outs, axis=0)  # [64, 4096]
    return out.reshape(1, 64, 64, 64)
